# revision 33
# baseline (speedup 1.0000x reference)
"""Causal self-attention on 8 TRN2 NeuronCores.

Sharding: core c = (batch b = c//2, head-group g = c%2).  Each core computes
the full attention for one batch and 8 of the 16 heads (column-sharded
Wq/Wk/Wv, row-sharded Wproj), producing a partial output projection; the two
partials per batch are summed on the host (the row-parallel all-reduce).

v4 schedule (baseline v1 was ~146us):
  - Host pre-arranges every input so each DMA is one large transfer with
    contiguous multi-KB per-partition lines; loads issue in compute-need
    order on the sync HWDGE ring.  (v1's 36 small interleaved DMAs completed
    their first chunk at ~9.5us and PE idled until ~12.6us.)
  - ~30 warm-up matmuls on a zeroed tile run right after the engine preamble
    (~6us) so the PE HAM clock gate opens (2.4GHz) before real work and stays
    open through the DMA-limited lead-in; a dummy psum read guards them from
    DCE.
  - Q/K for pair 0 are computed kt-major across the 4 main psum slots so the
    PE starts on each x/W chunk as it lands; all other QKV groups and the
    output projection are pumped as fillers between attention steps.
  - Attention runs as 2 interleaved (pair, q-half) units (qt=1 pairs then
    qt=0 pairs, staggered), so one unit's exp/mask latency is hidden by the
    other's matmuls plus fillers; per-step score tiles rotate the 4-slot
    psum pool so consecutive steps never serialize on one exp.
  - qt=1 projection groups unlock as soon as the 4th qt=1 tail is emitted,
    feeding the filler-starved qt=0 phase; two are held in reserve for the
    final drain window behind the last tail's SBUF->SBUF DMA.
"""

import numpy as np
import ml_dtypes
from contextlib import ExitStack

import concourse.tile as tile
from concourse import bacc, mybir
from concourse.bass import ts
from concourse.bass_utils import run_bass_kernel_spmd

F32 = mybir.dt.float32
BF16 = mybir.dt.bfloat16
AF = mybir.ActivationFunctionType

N_CORES = 8
T = 1024
C = 1024
D = 64          # head dim
HL = 8          # heads per core
CL = HL * D     # 512 local channels
NKT = 8         # contraction / key tiles of 128
NPAIR = 4       # head pairs per core
N_DUMMY = 30    # HAM warm-up matmuls (~8 cold + 22 warm ≈ 8us of cover)

_CACHE = {}


def _build():
    nc = bacc.Bacc("TRN2", target_bir_lowering=False, debug=False,
                   num_devices=N_CORES)
    # host-prearranged: partition dim first, contiguous per-partition lines
    xt = nc.dram_tensor("xt", [128, NKT, T], BF16, kind="ExternalInput").ap()
    wq = nc.dram_tensor("wq", [128, NPAIR, NKT, 128], BF16,
                        kind="ExternalInput").ap()
    wk = nc.dram_tensor("wk", [128, NPAIR, NKT, 128], BF16,
                        kind="ExternalInput").ap()
    wv = nc.dram_tensor("wv", [128, NKT, CL], BF16, kind="ExternalInput").ap()
    wp = nc.dram_tensor("wp", [128, NPAIR, C], BF16, kind="ExternalInput").ap()
    # cst cols 0:256 = [tri|tri] (keep where query>=key, one copy per head of
    # the pair); col block 256:320 row 64 is the ones row for the sums bcast
    cst = nc.dram_tensor("cst", [128, 384], BF16, kind="ExternalInput").ap()
    y = nc.dram_tensor("y", [T, C], BF16, kind="ExternalOutput").ap()

    with tile.TileContext(nc) as tc, ExitStack() as ctx:
        big = ctx.enter_context(tc.tile_pool(name="big", bufs=1))
        ps = ctx.enter_context(tc.tile_pool(name="ps", bufs=4, space="PSUM"))
        ps_out = ctx.enter_context(tc.tile_pool(name="ps_out", bufs=2,
                                                space="PSUM"))
        sb_e = ctx.enter_context(tc.tile_pool(name="sb_e", bufs=8))
        sb_t = ctx.enter_context(tc.tile_pool(name="sb_t", bufs=3))
        sb_y = ctx.enter_context(tc.tile_pool(name="sb_y", bufs=4))

        qT_sb = big.tile([128, NPAIR, T], BF16)
        kT_sb = big.tile([128, NPAIR, T], BF16)
        v_sb = big.tile([128, NKT, HL, D + 1], BF16)
        projT_sb = big.tile([128, NPAIR, T], BF16)

        # ---- warm-up: PE busy right after preamble so HAM un-throttles ----
        warm = big.tile([128, 512], BF16)
        nc.vector.memset(warm[:], 0.0)
        dps = ps.tile([128, 512], F32, tag="ps", name="dps")
        for _ in range(N_DUMMY):
            nc.tensor.matmul(dps[:], warm[:, 0:128], warm[:],
                             start=True, stop=True)
        # keep the dummies alive through DCE; overwritten by the tail muls
        nc.vector.tensor_copy(projT_sb[0:1, 0, 0:1], dps[0:1, 0:1])

        # ---- loads: one ring, compute-need order ----
        cst_sb = big.tile([128, 384], BF16)
        nc.sync.dma_start(out=cst_sb[:], in_=cst)
        xt_sb = big.tile([128, NKT, T], BF16)
        wq_sb = big.tile([128, NPAIR, NKT, 128], BF16)
        wk_sb = big.tile([128, NPAIR, NKT, 128], BF16)
        wv_sb = big.tile([128, NKT, CL], BF16)
        wp_sb = big.tile([128, NPAIR, C], BF16)
        nc.sync.dma_start(out=xt_sb[:, 0:2], in_=xt[:, 0:2])
        nc.sync.dma_start(out=wq_sb[:, 0], in_=wq[:, 0])
        nc.sync.dma_start(out=wk_sb[:, 0], in_=wk[:, 0])
        nc.sync.dma_start(out=xt_sb[:, 2:4], in_=xt[:, 2:4])
        nc.sync.dma_start(out=xt_sb[:, 4:6], in_=xt[:, 4:6])
        nc.sync.dma_start(out=xt_sb[:, 6:8], in_=xt[:, 6:8])
        nc.sync.dma_start(out=wv_sb[:, 0:4], in_=wv[:, 0:4])
        nc.sync.dma_start(out=wq_sb[:, 1], in_=wq[:, 1])
        nc.sync.dma_start(out=wk_sb[:, 1], in_=wk[:, 1])
        nc.sync.dma_start(out=wv_sb[:, 4:8], in_=wv[:, 4:8])
        nc.sync.dma_start(out=wq_sb[:, 2], in_=wq[:, 2])
        nc.sync.dma_start(out=wk_sb[:, 2], in_=wk[:, 2])
        nc.sync.dma_start(out=wq_sb[:, 3], in_=wq[:, 3])
        nc.sync.dma_start(out=wk_sb[:, 3], in_=wk[:, 3])
        nc.sync.dma_start(out=wp_sb[:], in_=wp)

        # v ones column: attn@[v|1] accumulates softmax sums at psum row 64.
        # memset the whole tile (contiguous); v copies overwrite cols 0:64.
        nc.vector.memset(v_sb[:], 1.0)

        # ---- prologue: Q/K pair 0, kt-major over the 4 main psum slots ----
        pw = [ps.tile([128, 512], F32, tag="ps", name=f"pw{i}")
              for i in range(4)]
        for kt in range(NKT):
            for gi, (w_sb, nt) in enumerate(
                    ((wq_sb, 0), (wq_sb, 1), (wk_sb, 0), (wk_sb, 1))):
                nc.tensor.matmul(
                    pw[gi][:], w_sb[:, 0, kt, :], xt_sb[:, kt, ts(nt, 512)],
                    start=(kt == 0), stop=(kt == NKT - 1))
        for gi, (dst, nt) in enumerate(
                ((qT_sb, 0), (qT_sb, 1), (kT_sb, 0), (kT_sb, 1))):
            nc.vector.tensor_copy(dst[:, 0, ts(nt, 512)], pw[gi][:])

        # ---- filler generators ----
        def wave_group(qk, m, nt):
            w_sb = wq_sb if qk == "q" else wk_sb
            dst = qT_sb if qk == "q" else kT_sb
            wps = ps.tile([128, 512], F32, tag="ps", name="wps")
            for kt in range(NKT):
                nc.tensor.matmul(
                    wps[:], w_sb[:, m, kt, :], xt_sb[:, kt, ts(nt, 512)],
                    start=(kt == 0), stop=(kt == NKT - 1))
                if kt % 2 == 1:
                    yield
            nc.vector.tensor_copy(dst[:, m, ts(nt, 512)], wps[:])

        def v_group(tt):
            vps = ps.tile([128, 512], F32, tag="ps", name="vps")
            for kt in range(NKT):
                nc.tensor.matmul(
                    vps[:], xt_sb[:, kt, ts(tt, 128)], wv_sb[:, kt, :],
                    start=(kt == 0), stop=(kt == NKT - 1))
                if kt % 2 == 1:
                    yield
            nc.vector.tensor_copy(
                v_sb[:, tt, :, 0:D],
                vps[:].rearrange("p (h d) -> p h d", h=HL))

        def proj_group(q0, tt2, n2):
            pps = ps.tile([128, 512], F32, tag="ps", name="pps")
            for r in range(NPAIR):
                nc.tensor.matmul(
                    pps[:],
                    projT_sb[:, r, q0 + 128 * tt2:q0 + 128 * (tt2 + 1)],
                    wp_sb[:, r, ts(n2, 512)],
                    start=(r == 0), stop=(r == NPAIR - 1))
                if r % 2 == 1:
                    yield
            yt = sb_y.tile([128, 512], BF16)
            nc.vector.tensor_copy(yt[:], pps[:])
            nc.sync.dma_start(
                out=y[q0 + 128 * tt2:q0 + 128 * (tt2 + 1), ts(n2, 512)],
                in_=yt[:])

        fillers = []  # [tag, generator]

        def pump(n):
            while n > 0 and fillers:
                tag, g = fillers[0]
                try:
                    next(g)
                    n -= 1
                except StopIteration:
                    fillers.pop(0)

        def flush(tags):
            i = 0
            while i < len(fillers):
                tag, g = fillers[i]
                if tag in tags:
                    for _ in g:
                        pass
                    fillers.pop(i)
                else:
                    i += 1

        for tt in range(NKT):
            fillers.append((f"v{tt}", v_group(tt)))
        for m in (1, 2, 3):
            for qk in ("q", "k"):
                for nt in range(2):
                    fillers.append((f"w{m}", wave_group(qk, m, nt)))

        tails_done = {0: 0, 1: 0}
        reserved = []

        # ---- attention units ----
        class Unit:
            def __init__(self, m, qt):
                self.m, self.qt = m, qt
                self.q0 = 512 * qt
                self.kts = list(range(4 * qt + 4))
                self.i = 0
                self.outAB = ps_out.tile([65, 2, 512], F32, tag="out",
                                         name="outAB")
                self.pend = None

            def attnv(self, pend):
                e, kt, off = pend
                # ensure v_group(kt)'s copy is EMITTED before this read,
                # else Tile misses the RAW dep (emit-order race)
                flush({f"v{kt}"})
                first, last = kt == self.kts[0], kt == self.kts[-1]
                for hh in range(2):
                    nc.tensor.matmul(
                        self.outAB[0:65, hh, off:512],
                        v_sb[:, kt, 2 * self.m + hh, 0:65],
                        e[:, hh, 0:512 - off],
                        start=first, stop=last)

            def step(self):
                # returns True while work remains
                if self.i < len(self.kts):
                    kt = self.kts[self.i]
                    off = max(0, 128 * kt - self.q0)
                    w = 512 - off
                    qcols = slice(self.q0 + off, self.q0 + 512)
                    sAB = [ps.tile([128, 512], F32, tag="ps", name="sc")
                           for _ in range(2)]
                    for hh, po in ((0, 0), (1, 64)):
                        nc.tensor.matmul(
                            sAB[hh][:, :w],
                            kT_sb[po:po + 64, self.m, ts(kt, 128)],
                            qT_sb[po:po + 64, self.m, qcols],
                            start=True, stop=True, tile_position=(po, 0))
                    e = sb_e.tile([128, 2, 512], BF16, name="et")
                    for hh in range(2):
                        nc.scalar.activation(e[:, hh, :w], sAB[hh][:, :w],
                                             AF.Exp, scale=0.125)
                    if kt >= 4 * self.qt:  # diagonal: zero upper triangle
                        # one fused DVE mul over both heads keeps the
                        # exp->mask->attnV chain short
                        nc.vector.tensor_mul(
                            e[:, :, 0:128], e[:, :, 0:128],
                            cst_sb[:, 0:256].rearrange(
                                "p (b f) -> p b f", b=2))
                    prev, self.pend = self.pend, (e, kt, off)
                    if prev is not None:
                        self.attnv(prev)
                    self.i += 1
                    return True
                if self.pend is not None:
                    self.attnv(self.pend)
                    self.pend = None
                    self.tail()
                return False

            def tail(self):
                m, q0 = self.m, self.q0
                rr = sb_t.tile([65, 2, 512], BF16, tag="rr", name="rr")
                for hh in range(2):
                    nc.vector.tensor_copy(rr[64:65, hh, :],
                                          self.outAB[64:65, hh, :])
                pump(2)
                bcs = []
                for hh in range(2):
                    bc = ps.tile([64, 512], F32, tag="ps", name="bc")
                    nc.tensor.matmul(
                        bc[:], cst_sb[64:65, ts(4, 64)], rr[64:65, hh, :],
                        start=True, stop=True, tile_position=(64, 0))
                    bcs.append(bc)
                pump(1)
                for hh in range(2):
                    bcr = sb_t.tile([64, 512], F32, tag="bcr", name="bcr")
                    nc.vector.reciprocal_approx_fast(out=bcr[:],
                                                     in_=bcs[hh][:])
                    if hh == 0:
                        nc.vector.tensor_mul(
                            projT_sb[0:64, m, q0:q0 + 512],
                            self.outAB[0:64, 0, :], bcr[:])
                    else:
                        t2 = sb_t.tile([64, 512], BF16, tag="t2", name="t2")
                        nc.vector.tensor_mul(
                            t2[:], self.outAB[0:64, 1, :], bcr[:])
                        if (m, self.qt) == (NPAIR - 1, 0):
                            # last tail gates the final proj groups: shift
                            # partitions via PE identity (~0.5us) instead of
                            # an SBUF->SBUF DMA (~2.5us receipt latency)
                            psh = ps.tile([128, 512], F32, tag="ps",
                                          name="psh")
                            nc.tensor.matmul(
                                psh[64:128, :], cst_sb[0:64, 320:384],
                                t2[:], start=True, stop=True,
                                tile_position=(0, 64))
                            nc.vector.tensor_copy(
                                projT_sb[64:128, m, q0:q0 + 512],
                                psh[64:128, :])
                        else:
                            # scalar ring: don't queue behind y writes
                            nc.scalar.dma_start(
                                out=projT_sb[64:128, m, q0:q0 + 512],
                                in_=t2[:])
                tails_done[self.qt] += 1
                if self.qt == 1 and tails_done[1] == NPAIR:
                    for tt2 in range(4):
                        for n2 in range(2):
                            g = proj_group(512, tt2, n2)
                            if tt2 >= 2:
                                # held back: dense PE cover for the final
                                # tail's SBUF->SBUF shift DMA window
                                reserved.append(("p1r", g))
                            else:
                                fillers.append(("p1", g))
                if self.qt == 0 and tails_done[0] == NPAIR:
                    fillers.extend(reserved)
                    for tt2 in range(4):
                        for n2 in range(2):
                            fillers.append(("p0", proj_group(0, tt2, n2)))

        # unit order: qt=1 pairs then qt=0 pairs; 2 active, staggered
        order = [(m, 1) for m in range(NPAIR)] + [(m, 0) for m in range(NPAIR)]
        stagger = {1: 4, 0: 1}
        active = []
        pending = list(order)

        def admit_ok():
            if not pending or len(active) >= 2:
                return False
            if active and active[-1].i < stagger[pending[0][1]]:
                return False
            return True

        while pending or active:
            while admit_ok():
                m, qt = pending.pop(0)
                flush({f"w{m}"})
                active.append(Unit(m, qt))
            for u in list(active):
                if not u.step():
                    active.remove(u)
                pump(2)

        while fillers:
            pump(len(fillers) * 8)

    nc.compile()
    return nc


def _program():
    if "nc" not in _CACHE:
        _CACHE["nc"] = _build()
    return _CACHE["nc"]


def _bf(a):
    return np.ascontiguousarray(a).astype(ml_dtypes.bfloat16)


def _in_maps(x, Wq, Wk, Wv, Wproj):
    tri = np.triu(np.ones((128, 128), dtype=np.float32))  # keep f >= p
    sel = np.zeros((128, 128), dtype=np.float32)
    sel[64, 0:64] = 1.0   # ones row for the sums broadcast matmul
    sel[0:64, 64:128] = np.eye(64, dtype=np.float32)  # last-tail shift
    cst = _bf(np.concatenate([tri, tri, sel], axis=1))
    maps = []
    for c in range(N_CORES):
        b, g = c // 2, c % 2
        sl = slice(CL * g, CL * (g + 1))
        # [p, kt, t]: xt[p, kt, t] = x[b].T[kt*128+p, t]
        xt_h = x[b].T.reshape(NKT, 128, T).transpose(1, 0, 2)
        # [p, m, kt, j]: wq[p, m, kt, j] = Wq[kt*128+p, 512*g + 128*m + j]
        wq_h = Wq[:, sl].reshape(NKT, 128, NPAIR, 128).transpose(1, 2, 0, 3)
        wk_h = Wk[:, sl].reshape(NKT, 128, NPAIR, 128).transpose(1, 2, 0, 3)
        # [p, kt, cl]
        wv_h = Wv[:, sl].reshape(NKT, 128, CL).transpose(1, 0, 2)
        # [p, r, n]: wp[p, r, n] = Wproj[512*g + r*128 + p, n]
        wp_h = Wproj[sl, :].reshape(NPAIR, 128, C).transpose(1, 0, 2)
        maps.append({
            "xt": _bf(xt_h), "wq": _bf(wq_h), "wk": _bf(wk_h),
            "wv": _bf(wv_h), "wp": _bf(wp_h), "cst": cst,
        })
    return maps


def run(x, Wq, Wk, Wv, Wproj, trace=False, **kwargs):
    nc = _program()
    maps = _in_maps(np.asarray(x, dtype=np.float32),
                    np.asarray(Wq, dtype=np.float32),
                    np.asarray(Wk, dtype=np.float32),
                    np.asarray(Wv, dtype=np.float32),
                    np.asarray(Wproj, dtype=np.float32))
    res = run_bass_kernel_spmd(nc, maps, core_ids=list(range(N_CORES)),
                               trace=trace, **kwargs)
    B = 4
    out = np.empty((B, T, C), dtype=np.float32)
    for b in range(B):
        out[b] = (res.results[2 * b]["y"].astype(np.float32)
                  + res.results[2 * b + 1]["y"].astype(np.float32))
    return out, res


def kernel(x, Wq, Wk, Wv, Wproj):
    out, _ = run(x, Wq, Wk, Wv, Wproj)
    return out
